# revision 2
# baseline (speedup 1.0000x reference)
"""TopoEncoder Trainium2 kernel (8 NeuronCores, data-parallel over batch).

Two-pass design — the reference's single global scalar (max over the whole
batch's distance tensor) is the only cross-core quantity, and the CC-stream
collective path costs ~65us of pure latency on these axon-tunneled cores
(43.6us kernel-entry barrier + 11us stream gap + 9.5us mesh op). Instead:

  pass 1 (per core, 64 samples): x DMA -> mean over T (DVE/GpSimd add-tree +
    PE pair-matrix fold) -> pairwise channel-L2 distance^2 -> fp16 ->
    Floyd-Warshall min-max closure in fp16 (selection ops only, so fp16
    rounds each d^2 once; measured output rel err ~7e-6) -> MST mask
    (M >= d, exact on fp16 values) -> top-24 masked upper-tri extraction
    (max8 + match_replace) -> sqrt -> deaths [64,24] f32, plus the
    per-sample max of d^2 [64,1].
  host: folds gmax = sqrt(max of the 512 per-sample maxima) into the
    structure-element parameters (pure parameter prep, like the baseline's
    csT/pairmat): C_e = 1e-6 + c2_e*R, U_e = (s2_e/R)^2, Ab_e = exp(-(s1 c1)^2)
    with R = gmax - 1e-6 (global min is the d=sqrt(1e-12) diagonal,
    structurally 1e-6).
  pass 2 (per core): normalize-free structure-element layer on deaths with
    the folded params: out[b,e] = Ab_e * sum_p exp(-U_e (death_p - C_e)^2).

fp16 matters because DVE's 2x mode needs 2-byte dtypes with packed innermost
access; the FW min and the mask/extraction all hit it (the col-broadcast max
stays at 1x - access-pattern-bound, not ALU-bound).
"""

from contextlib import ExitStack

import numpy as np

import bass_rust
import concourse.bass as bass
import concourse.tile as tile
from concourse import mybir
from concourse.bass_utils import run_bass_kernel_spmd

N_CORES = 8
B = 64          # samples per core
C, T, V, E = 3, 128, 25, 64
VV = V * V
NT = V - 1      # deaths per sample (24)
DT = mybir.dt.float32
F16 = mybir.dt.float16
GMIN = 1e-6     # sqrt(1e-12): the reference's global min (diagonal), exact


def _split_excess_waits(nc, cap=1):
    """The walrus build in this env rejects instructions carrying more than
    ~2 semaphore-wait commands. Move excess waits onto same-engine NOPs
    inserted immediately before the offending instruction."""
    n_split = 0
    for bb in nc.main_func.blocks:
        insts = bb.instructions
        i = 0
        while i < len(insts):
            ins = insts[i]
            si = ins.sync_info
            waits = list(si.on_wait) if si and si.on_wait else []
            if len(waits) > cap:
                extra, keep = waits[:-cap], waits[-cap:]
                ins.sync_info = mybir.SyncInfo(
                    on_wait=keep, on_update=list(si.on_update or [])
                )
                for j, w in enumerate(extra):
                    nop = bass_rust.InstNoOp(
                        name=f"I-wsplit-{n_split}-{j}",
                        engine=ins.engine,
                        sync_info=mybir.SyncInfo(on_wait=[w], on_update=[]),
                    )
                    insts.insert(i, nop)
                    i += 1
                n_split += 1
            i += 1
    return n_split


def _build_pass1():
    A = mybir.AluOpType
    ACT = mybir.ActivationFunctionType
    nc = bass.Bass("TRN2", debug=False, num_devices=N_CORES)

    x_in = nc.dram_tensor("x", [B, C, T, V], DT, kind="ExternalInput").ap()
    pm_in = nc.dram_tensor("pm", [128, B], DT, kind="ExternalInput").ap()
    ut_in = nc.dram_tensor("ut", [1, VV], F16, kind="ExternalInput").ap()
    dth_d = nc.dram_tensor("deaths", [B, NT], DT, kind="ExternalOutput").ap()
    pmx_d = nc.dram_tensor("pmax", [B, 1], F16, kind="ExternalOutput").ap()

    with tile.TileContext(nc, num_cores=N_CORES) as tc, ExitStack() as ctx:
        sb = ctx.enter_context(tc.tile_pool(name="sb", bufs=1))
        psum = ctx.enter_context(tc.tile_pool(name="psum", bufs=1, space="PSUM"))

        # ---- x DMA first: partition p = t2*64 + b, free = (c, t32, v) ----
        xa = sb.tile([128, C, T // 4, V], DT)
        xb = sb.tile([128, C, T // 4, V], DT)
        nc.sync.dma_start(xa[0:B], x_in[:, :, 0:32, :])
        nc.scalar.dma_start(xa[B:128], x_in[:, :, 64:96, :])
        nc.sync.dma_start(xb[0:B], x_in[:, :, 32:64, :])
        nc.scalar.dma_start(xb[B:128], x_in[:, :, 96:128, :])

        # ---- small constant loads ----
        pm_t = sb.tile([128, B], DT)
        nc.sync.dma_start(pm_t[:], pm_in[:])
        utrow = sb.tile([1, VV], F16)
        nc.scalar.dma_start(utrow[:], ut_in[:])
        ones1 = sb.tile([1, B], F16)
        nc.vector.memset(ones1[:], 1.0)

        # ---- PE partition-broadcast of the upper-tri premask row ----
        utb = psum.tile([B, VV], DT)
        nc.tensor.matmul(out=utb[:, 0:512], lhsT=ones1[:], rhs=utrow[:, 0:512],
                         start=True, stop=True)
        nc.tensor.matmul(out=utb[:, 512:VV], lhsT=ones1[:], rhs=utrow[:, 512:VV],
                         start=True, stop=True)

        # ---- mean over T: add trees (DVE: c0-c1, GpSimd: c2), PE pair fold ----
        for xh in (xa, xb):
            for w in (16, 8, 4, 2, 1):
                nc.vector.tensor_tensor(
                    out=xh[:, 0:2, 0:w, :],
                    in0=xh[:, 0:2, 0:w, :],
                    in1=xh[:, 0:2, w : 2 * w, :],
                    op=A.add,
                )
                nc.gpsimd.tensor_tensor(
                    out=xh[:, 2, 0:w, :],
                    in0=xh[:, 2, 0:w, :],
                    in1=xh[:, 2, w : 2 * w, :],
                    op=A.add,
                )
        nc.vector.tensor_tensor(
            out=xa[:, 0:2, 0:1, :], in0=xa[:, 0:2, 0:1, :], in1=xb[:, 0:2, 0:1, :],
            op=A.add,
        )
        nc.gpsimd.tensor_tensor(
            out=xa[:, 2, 0:1, :], in0=xa[:, 2, 0:1, :], in1=xb[:, 2, 0:1, :],
            op=A.add,
        )
        ps_xm = psum.tile([B, C, V], DT)
        nc.tensor.matmul(out=ps_xm[:], lhsT=pm_t[:], rhs=xa[:, :, 0, :],
                         start=True, stop=True)
        xm = sb.tile([B, C, V], DT)
        nc.vector.tensor_copy(xm[:], ps_xm[:])

        # ---- distance^2 matrix (no sqrt needed before the deaths) ----
        df = sb.tile([B, C, V, V], DT)
        xmb_i = xm.unsqueeze(-1).broadcast_to([B, C, V, V])
        xmb_j = xm.unsqueeze(2).broadcast_to([B, C, V, V])
        nc.vector.tensor_tensor(
            out=df[:, 0:2], in0=xmb_i[:, 0:2], in1=xmb_j[:, 0:2], op=A.subtract
        )
        nc.gpsimd.tensor_tensor(
            out=df[:, 2], in0=xmb_i[:, 2], in1=xmb_j[:, 2], op=A.subtract
        )
        nc.scalar.square(df[:, 0:2], df[:, 0:2])
        nc.vector.tensor_tensor(out=df[:, 2], in0=df[:, 2], in1=df[:, 2], op=A.mult)
        d12 = sb.tile([B, VV], DT)
        d123 = d12.rearrange("p (i j) -> p i j", i=V)
        nc.vector.tensor_tensor(out=d123[:], in0=df[:, 0], in1=df[:, 1], op=A.add)
        dq = sb.tile([B, VV], F16)
        dq3 = dq.rearrange("p (i j) -> p i j", i=V)
        nc.vector.tensor_tensor(out=dq3[:], in0=d123[:], in1=df[:, 2], op=A.add)

        # ---- per-sample max of d^2 (host folds into the global max) ----
        pmx = sb.tile([B, 1], F16)
        nc.vector.tensor_reduce(out=pmx[:], in_=dq[:],
                                axis=mybir.AxisListType.X, op=A.max)
        nc.scalar.dma_start(pmx_d[:], pmx[:])

        # ---- premasked values (upper-tri), overlaps nothing critical ----
        dut = sb.tile([B, VV], F16)
        nc.vector.tensor_tensor(out=dut[:], in0=dq[:], in1=utb[:], op=A.mult)

        # ---- Floyd-Warshall min-max closure in fp16 (selection-only) ----
        M = sb.tile([B, VV], F16)
        nc.vector.tensor_copy(M[:], dq[:])
        M3 = M.rearrange("p (i j) -> p i j", i=V)
        fwt = sb.tile([B, V, V], F16)
        for k in range(V):
            nc.vector.tensor_tensor(
                out=fwt[:],
                in0=M3[:, :, k : k + 1].broadcast_to([B, V, V]),
                in1=M3[:, k : k + 1, :].broadcast_to([B, V, V]),
                op=A.max,
            )
            nc.vector.tensor_tensor(out=M3[:], in0=M3[:], in1=fwt[:], op=A.min)

        # ---- MST mask + masked upper-tri values ----
        mk = sb.tile([B, VV], F16)
        nc.vector.tensor_tensor(out=mk[:], in0=M[:], in1=dq[:], op=A.is_ge)
        val = sb.tile([B, VV], F16)
        nc.vector.tensor_tensor(out=val[:], in0=mk[:], in1=dut[:], op=A.mult)

        # ---- extract 24 MST weights: 3 rounds of top-8 + match_replace ----
        d16 = sb.tile([B, NT], F16)
        mr1 = sb.tile([B, VV], F16)
        mr2 = sb.tile([B, VV], F16)
        nc.vector.max(d16[:, 0:8], val[:])
        nc.vector.match_replace(mr1[:], d16[:, 0:8], val[:], 0.0)
        nc.vector.max(d16[:, 8:16], mr1[:])
        nc.vector.match_replace(mr2[:], d16[:, 8:16], mr1[:], 0.0)
        nc.vector.max(d16[:, 16:24], mr2[:])

        # ---- deaths = sqrt(selected d^2), fp32 out ----
        dth = sb.tile([B, NT], DT)
        nc.scalar.activation(dth[:], d16[:], ACT.Sqrt, bias=0.0, scale=1.0)
        nc.sync.dma_start(dth_d[:], dth[:])

    _split_excess_waits(nc)
    return nc


def _build_pass2():
    A = mybir.AluOpType
    ACT = mybir.ActivationFunctionType
    nc = bass.Bass("TRN2", debug=False, num_devices=N_CORES)

    dth_in = nc.dram_tensor("deaths", [B, NT], DT, kind="ExternalInput").ap()
    prm_in = nc.dram_tensor("prm", [1, 3 * E], DT, kind="ExternalInput").ap()
    out_d = nc.dram_tensor("out", [B, E], DT, kind="ExternalOutput").ap()

    with tile.TileContext(nc, num_cores=N_CORES) as tc, ExitStack() as ctx:
        sb = ctx.enter_context(tc.tile_pool(name="sb", bufs=1))
        work = ctx.enter_context(tc.tile_pool(name="work", bufs=2))
        psum = ctx.enter_context(tc.tile_pool(name="psum", bufs=1, space="PSUM"))

        dth = sb.tile([B, NT], DT)
        nc.sync.dma_start(dth[:], dth_in[:])
        prow = sb.tile([1, 3 * E], DT)
        nc.scalar.dma_start(prow[:], prm_in[:])
        ones1 = sb.tile([1, B], DT)
        nc.vector.memset(ones1[:], 1.0)

        # broadcast params to all partitions: [B, 3, E] = (C_e, U_e, Ab_e)
        prm = psum.tile([B, 3, E], DT)
        nc.tensor.matmul(out=prm[:], lhsT=ones1[:], rhs=prow[:], start=True, stop=True)
        Cb = sb.tile([B, E], DT)
        nc.vector.tensor_copy(Cb[:], prm[:, 0, :])
        Ub = sb.tile([B, E], DT)
        nc.vector.tensor_copy(Ub[:], prm[:, 1, :])
        Ab = sb.tile([B, E], DT)
        nc.vector.tensor_copy(Ab[:], prm[:, 2, :])

        # structure element layer: out[b,e] = Ab_e * sum_p exp(-U_e (dth - C_e)^2)
        S = sb.tile([B, E], DT)
        ECH = 32
        for ch in range(E // ECH):
            e0 = ch * ECH
            t1 = work.tile([B, ECH, NT], DT, tag="t1")
            nc.vector.tensor_tensor(
                out=t1[:],
                in0=dth.unsqueeze(1).broadcast_to([B, ECH, NT]),
                in1=Cb[:, e0 : e0 + ECH].unsqueeze(-1).broadcast_to([B, ECH, NT]),
                op=A.subtract,
            )
            nc.scalar.square(t1[:], t1[:])
            nc.vector.tensor_tensor(
                out=t1[:],
                in0=t1[:],
                in1=Ub[:, e0 : e0 + ECH].unsqueeze(-1).broadcast_to([B, ECH, NT]),
                op=A.mult,
            )
            fexp = work.tile([B, ECH, NT], DT, tag="fexp")
            nc.scalar.activation(fexp[:], t1[:], ACT.Exp, bias=0.0, scale=-1.0)
            nc.vector.tensor_reduce(
                out=S[:, e0 : e0 + ECH], in_=fexp[:], axis=mybir.AxisListType.X,
                op=A.add,
            )
        outt = sb.tile([B, E], DT)
        nc.vector.tensor_tensor(out=outt[:], in0=S[:], in1=Ab[:], op=A.mult)
        nc.sync.dma_start(out_d[:], outt[:])

    _split_excess_waits(nc)
    return nc


_CACHE = {}


def _consts():
    # pair matrix: adds partition rows b and b+64 (the two T-halves) and
    # applies the 1/T mean scale
    pairmat = np.zeros((128, B), dtype=np.float32)
    for p in range(128):
        pairmat[p, p % B] = 1.0 / T
    ut = np.triu(np.ones((V, V), dtype=np.float16), k=1).reshape(1, VV)
    return pairmat, np.ascontiguousarray(ut)


def _get_programs():
    if "p1" not in _CACHE:
        _CACHE["p1"] = _build_pass1()
        _CACHE["p2"] = _build_pass2()
    return _CACHE["p1"], _CACHE["p2"]


def _run(x, centres, sharpness, **run_kwargs):
    p1, p2 = _get_programs()
    xf = np.ascontiguousarray(x.reshape(-1, C, T, V)).astype(np.float32, copy=False)
    n_total = xf.shape[0]
    assert n_total == N_CORES * B, xf.shape
    pairmat, ut = _consts()

    in1 = [
        {"x": np.ascontiguousarray(xf[i * B : (i + 1) * B]), "pm": pairmat, "ut": ut}
        for i in range(N_CORES)
    ]
    res1 = run_bass_kernel_spmd(p1, in1, list(range(N_CORES)), **run_kwargs)

    # host: fold the global max into the structure-element parameters
    gmax2 = max(
        float(np.max(res1.results[i]["pmax"].astype(np.float32)))
        for i in range(N_CORES)
    )
    gmax = float(np.sqrt(gmax2))
    R = gmax - GMIN
    c1 = centres[:, 0].astype(np.float64)
    c2 = centres[:, 1].astype(np.float64)
    s1 = sharpness[:, 0].astype(np.float64)
    s2 = sharpness[:, 1].astype(np.float64)
    Ce = GMIN + c2 * R
    Ue = (s2 / R) ** 2
    Abe = np.exp(-((s1 * c1) ** 2))
    prm = np.ascontiguousarray(
        np.stack([Ce, Ue, Abe], axis=0).astype(np.float32).reshape(1, 3 * E)
    )

    in2 = [
        {"deaths": np.ascontiguousarray(res1.results[i]["deaths"]), "prm": prm}
        for i in range(N_CORES)
    ]
    res2 = run_bass_kernel_spmd(p2, in2, list(range(N_CORES)), **run_kwargs)

    out = np.concatenate([res2.results[i]["out"] for i in range(N_CORES)], axis=0)
    return out, (res1, res2)


def kernel(x, centres, sharpness):
    out, _ = _run(np.asarray(x), np.asarray(centres), np.asarray(sharpness))
    return out
